# revision 6
# baseline (speedup 1.0000x reference)
"""Trainium2 Bass kernel for Bahdanau-style attention.

reference:
    x[s,b,u]  = enc[s,b,:] @ Ua_w[u,:] + Ua_b[u] + dec[b,:] @ Wa_w[u,:] + Wa_b[u]
    att[b,s]  = softmax_s( sum_u v[u] * tanh(x[s,b,u]) )

Sharding: data-parallel over batch. 8 cores x 8 batches each; weights
replicated. All shapes hardcoded (S=512, B=64, H=2048, U=1024).

Per-core device program (bf16 matmul operands, fp32 accumulation):
  1. D[b,u] = dec-projection with Ua_b+Wa_b folded in via an extra
     contraction row (dec.T stationary so weight loads are 8 columns),
     then PE-transposed to D_T[u, ut*8+b] so it can feed the ScalarE
     activation bias port.
  2. for each local batch b (rows are b-major so the dec projection is
     constant along the moving free dim):
       for each u-tile (8 of 128):
         psum[128u, 512s] = sum_hc UaT[hc,utile].T @ enc[hc, b-rows]
         energy = tanh(psum + D_T[:, col])        (ScalarE, bias port)
         acc   += energy * v[utile]               (DVE fused mul-add)
       att[1, 512] = ones.T @ acc                 (PE partition-sum)
       softmax over s on [1, 512], DMA out row b.
"""

import numpy as np
import ml_dtypes

BF16 = ml_dtypes.bfloat16

S = 512          # src len
B = 64           # global batch
H = 2048         # encoder hidden (2*HIDDEN)
HD = 1024        # decoder hidden
U = 1024         # attention units
NCORES = 8
BL = B // NCORES  # local batch per core = 8

HC = H // 128     # 16 h-chunks for main contraction
UT = U // 128     # 8 u-tiles
HA = HD + 128     # augmented dec contraction (1024 + bias row + pad) = 1152
HAC = HA // 128   # 9 chunks

_BUILT = None     # cache so repeated kernel() calls reuse the program


def _build_bass():
    import concourse.mybir as mybir
    from concourse import bacc
    from concourse.tile import TileContext
    from concourse.masks import make_identity

    f32 = mybir.dt.float32
    bf16 = mybir.dt.bfloat16
    Tanh = mybir.ActivationFunctionType.Tanh
    Exp = mybir.ActivationFunctionType.Exp
    X = mybir.AxisListType.X
    MULT = mybir.AluOpType.mult
    ADD = mybir.AluOpType.add

    nc = bacc.Bacc("TRN2", num_devices=NCORES)

    # p-major layouts: partition p's data is contiguous in DRAM, so DMAs
    # are 128 descriptors of multi-KB runs instead of thousands of 1KB rows
    enc_d = nc.dram_tensor("enc", [BL, 128, HC, S], bf16, kind="ExternalInput")
    uat_d = nc.dram_tensor("uat", [128, HC, U], bf16, kind="ExternalInput")
    wat_d = nc.dram_tensor("wat", [128, HAC, U], bf16, kind="ExternalInput")
    dect_d = nc.dram_tensor("dect", [128, HAC, BL], bf16, kind="ExternalInput")
    v_d = nc.dram_tensor("v", [128, UT], f32, kind="ExternalInput")
    out_d = nc.dram_tensor("out", [BL, S], f32, kind="ExternalOutput")

    with TileContext(nc) as tc:
        with (
            tc.tile_pool(name="const", bufs=1) as const,
            tc.tile_pool(name="encp", bufs=2) as encp,
            tc.tile_pool(name="energy", bufs=4) as energy,
            tc.tile_pool(name="accp", bufs=3) as accp,
            tc.tile_pool(name="smax", bufs=2) as smax,
            tc.tile_pool(name="psum_main", bufs=4, space="PSUM") as psum_main,
            tc.tile_pool(name="psum_v", bufs=2, space="PSUM") as psum_v,
            tc.tile_pool(name="psum_d", bufs=2, space="PSUM") as psum_d,
        ):
            # ---- constant loads; order = DMA issue order (startup path) ----
            dect_sb = const.tile([128, HAC, BL], bf16)
            nc.sync.dma_start(out=dect_sb, in_=dect_d[:, :, :])
            v_sb = const.tile([128, UT], f32)
            nc.sync.dma_start(out=v_sb, in_=v_d[:, :])
            wat_sb = const.tile([128, HAC, U], bf16)
            nc.sync.dma_start(out=wat_sb[:, 0:5, :], in_=wat_d[:, 0:5, :])
            nc.sync.dma_start(out=wat_sb[:, 5:HAC, :], in_=wat_d[:, 5:HAC, :])
            uat_sb = const.tile([128, HC, U], bf16)

            ones_sb = const.tile([128, 1], bf16)
            nc.vector.memset(ones_sb, 1.0)
            id8 = const.tile([8, 8], f32)
            make_identity(nc, id8)

            # ---- D = dec-projection (+folded biases), dec.T stationary ----
            # pdb[b, u-half] accumulated over 9 chunks; weight loads are the
            # tiny 8-column dec.T so PE setup cost is negligible.
            pdb0 = psum_d.tile([8, 512], f32, tag="pd")
            pdb1 = psum_d.tile([8, 512], f32, tag="pd")
            for hc in range(HAC):
                nc.tensor.matmul(
                    pdb0, lhsT=dect_sb[:, hc, :], rhs=wat_sb[:, hc, 0:512],
                    start=(hc == 0), stop=(hc == HAC - 1),
                )
            for hc in range(HAC):
                nc.tensor.matmul(
                    pdb1, lhsT=dect_sb[:, hc, :], rhs=wat_sb[:, hc, 512:1024],
                    start=(hc == 0), stop=(hc == HAC - 1),
                )
            dbu_sb = const.tile([8, U], f32)
            nc.vector.tensor_copy(out=dbu_sb[:, 0:512], in_=pdb0)
            nc.vector.tensor_copy(out=dbu_sb[:, 512:1024], in_=pdb1)
            d_sb = const.tile([128, UT * BL], f32)

            def emit_d_transposes():
                # D[b, u] -> D_T[u-in-tile, ut*8+b] via PE transpose
                pd = psum_d.tile([128, UT * BL], f32, tag="pd")
                for ut in range(UT):
                    nc.tensor.transpose(
                        pd[:, ut * BL:(ut + 1) * BL],
                        in_=dbu_sb[:, ut * 128:(ut + 1) * 128],
                        identity=id8,
                    )
                nc.vector.tensor_copy(out=d_sb, in_=pd)

            def emit_vdot_softmax(bl, acc):
                # att logits: partition-sum of acc via ones-matmul, then
                # per-row softmax entirely on partition 0
                pvs = psum_v.tile([1, S], f32)
                nc.tensor.matmul(pvs, lhsT=ones_sb, rhs=acc,
                                 start=True, stop=True)
                mneg = smax.tile([1, 1], f32)
                nc.vector.reduce_max(out=mneg, in_=pvs, axis=X, negate=True)
                ex = smax.tile([1, S], f32)
                ssum = smax.tile([1, 1], f32)
                nc.scalar.activation(out=ex, in_=pvs, func=Exp, bias=mneg,
                                     scale=1.0, accum_out=ssum)
                rsum = smax.tile([1, 1], f32)
                nc.vector.reciprocal(out=rsum, in_=ssum)
                res = smax.tile([1, S], f32)
                nc.vector.tensor_scalar_mul(res, ex, rsum)
                nc.sync.dma_start(out=out_d[bl:bl + 1, :], in_=res)

            # ---- main loop over local batches ----
            pending = None
            for bl in range(BL):
                enc_t = encp.tile([128, HC, S], bf16)
                for q in range(4):
                    h0, h1 = q * (HC // 4), (q + 1) * (HC // 4)
                    if bl == 0:
                        # interleave weight/activation streams so the first
                        # matmul's operands land early
                        nc.sync.dma_start(
                            out=uat_sb[:, h0:h1, :], in_=uat_d[:, h0:h1, :]
                        )
                    nc.sync.dma_start(
                        out=enc_t[:, h0:h1, :], in_=enc_d[bl, :, h0:h1, :]
                    )
                acc = None
                for ut in range(UT):
                    ps = psum_main.tile([128, S], f32)
                    for hc in range(HC):
                        nc.tensor.matmul(
                            ps,
                            lhsT=uat_sb[:, hc, ut * 128:(ut + 1) * 128],
                            rhs=enc_t[:, hc, :],
                            start=(hc == 0), stop=(hc == HC - 1),
                        )
                    if bl == 0 and ut == 0:
                        # PE reaches these after the first MM block, by which
                        # time the DVE psum->sbuf copies are long done
                        emit_d_transposes()
                    en = energy.tile([128, S], bf16)
                    col = ut * BL + bl
                    nc.scalar.activation(
                        out=en, in_=ps, func=Tanh,
                        bias=d_sb[:, col:col + 1], scale=1.0,
                    )
                    if ut == 0:
                        acc = accp.tile([128, S], bf16)
                        nc.vector.tensor_scalar_mul(acc, en, v_sb[:, 0:1])
                    else:
                        nc.vector.scalar_tensor_tensor(
                            out=acc, in0=en, scalar=v_sb[:, ut:ut + 1],
                            in1=acc, op0=MULT, op1=ADD,
                        )
                if pending is not None:
                    emit_vdot_softmax(*pending)
                pending = (bl, acc)
            emit_vdot_softmax(*pending)

    nc.finalize()
    return nc


def _get_nc():
    global _BUILT
    if _BUILT is None:
        _BUILT = _build_bass()
    return _BUILT


def _prep_inputs(encoder_hiddens, last_dec_hidden, Ua_w, Ua_b, Wa_w, Wa_b, v_w):
    """Host-side sharding + layout prep (transpose contraction dims onto
    partitions, cast matmul operands to bf16, fold biases into an extra
    contraction row)."""
    enc = np.asarray(encoder_hiddens, dtype=np.float32)
    dec = np.asarray(last_dec_hidden, dtype=np.float32)
    Ua_w = np.asarray(Ua_w, dtype=np.float32)
    Wa_w = np.asarray(Wa_w, dtype=np.float32)
    bias_u = (np.asarray(Ua_b, np.float32) + np.asarray(Wa_b, np.float32))

    # p-major: [S,B,H] -> [B, 128p, HC, S] bf16 so each partition's span
    # is contiguous in DRAM (one big permute copy; per-core slices are
    # then zero-copy views along axis 0)
    enc_pm = np.ascontiguousarray(
        enc.astype(BF16).transpose(2, 1, 0).reshape(HC, 128, B, S)
        .transpose(2, 1, 0, 3)
    )                                                          # [B,128,HC,S]

    uat = np.ascontiguousarray(
        Ua_w.T.reshape(HC, 128, U).transpose(1, 0, 2)
    ).astype(BF16)                                             # [128, HC, U]

    wat_aug = np.zeros((HA, U), np.float32)
    wat_aug[:HD] = Wa_w.T
    wat_aug[HD] = bias_u
    wat_aug = np.ascontiguousarray(
        wat_aug.reshape(HAC, 128, U).transpose(1, 0, 2)
    ).astype(BF16)                                             # [128, HAC, U]

    v_prep = np.ascontiguousarray(
        np.asarray(v_w, np.float32).reshape(UT, 128).T
    )                                                         # [128, UT] f32

    in_maps = []
    for c in range(NCORES):
        b0 = c * BL
        enc_c = enc_pm[b0:b0 + BL]                            # [BL,128,HC,S]

        dect_aug = np.zeros((HA, BL), np.float32)
        dect_aug[:HD] = dec[b0:b0 + BL, :].T
        dect_aug[HD] = 1.0
        dect_aug = np.ascontiguousarray(
            dect_aug.reshape(HAC, 128, BL).transpose(1, 0, 2)
        ).astype(BF16)                                        # [128,HAC,BL]

        in_maps.append({
            "enc": enc_c,
            "uat": uat,
            "wat": wat_aug,
            "dect": dect_aug,
            "v": v_prep,
        })
    return in_maps


def kernel_with_results(**inputs):
    from concourse.bass_utils import run_bass_kernel_spmd

    nc = _get_nc()
    in_maps = _prep_inputs(**inputs)
    res = run_bass_kernel_spmd(nc, in_maps, core_ids=list(range(NCORES)))
    out = np.concatenate(
        [res.results[c]["out"] for c in range(NCORES)], axis=0
    ).astype(np.float32)
    return out, res


def kernel(**inputs):
    out, _ = kernel_with_results(**inputs)
    return out


# revision 9
# speedup vs baseline: 1.0089x; 1.0089x over previous
"""Trainium2 Bass kernel for Bahdanau-style attention.

reference:
    x[s,b,u]  = enc[s,b,:] @ Ua_w[u,:] + Ua_b[u] + dec[b,:] @ Wa_w[u,:] + Wa_b[u]
    att[b,s]  = softmax_s( sum_u v[u] * tanh(x[s,b,u]) )

Sharding: data-parallel over batch. 8 cores x 8 batches each; weights
replicated. All shapes hardcoded (S=512, B=64, H=2048, U=1024).

Per-core device program (bf16 matmul operands, fp32 accumulation):
  1. D[b,u] = dec-projection with Ua_b+Wa_b folded in via an extra
     contraction row (dec.T stationary so weight loads are 8 columns),
     then PE-transposed to D_T[u, ut*8+b] so it can feed the ScalarE
     activation bias port.
  2. for each local batch b (rows are b-major so the dec projection is
     constant along the moving free dim):
       for each u-tile (8 of 128):
         psum[128u, 512s] = sum_hc UaT[hc,utile].T @ enc[hc, b-rows]
         energy = tanh(psum + D_T[:, col])        (ScalarE, bias port)
         acc   += energy * v[utile]               (DVE fused mul-add)
       att[1, 512] = ones.T @ acc                 (PE partition-sum)
       softmax over s on [1, 512], DMA out row b.
"""

import numpy as np
import ml_dtypes

BF16 = ml_dtypes.bfloat16

S = 512          # src len
B = 64           # global batch
H = 2048         # encoder hidden (2*HIDDEN)
HD = 1024        # decoder hidden
U = 1024         # attention units
NCORES = 8
BL = B // NCORES  # local batch per core = 8

HC = H // 128     # 16 h-chunks for main contraction
UT = U // 128     # 8 u-tiles
HA = HD + 128     # augmented dec contraction (1024 + bias row + pad) = 1152
HAC = HA // 128   # 9 chunks

_BUILT = None     # cache so repeated kernel() calls reuse the program


def _build_bass():
    import concourse.mybir as mybir
    from concourse import bacc
    from concourse.tile import TileContext
    from concourse.masks import make_identity

    f32 = mybir.dt.float32
    bf16 = mybir.dt.bfloat16
    Tanh = mybir.ActivationFunctionType.Tanh
    Exp = mybir.ActivationFunctionType.Exp
    X = mybir.AxisListType.X
    MULT = mybir.AluOpType.mult
    ADD = mybir.AluOpType.add

    nc = bacc.Bacc("TRN2", num_devices=NCORES)

    # p-major layouts: partition p's data is contiguous in DRAM, so DMAs
    # are 128 descriptors of multi-KB runs instead of thousands of 1KB rows
    enc_d = nc.dram_tensor("enc", [BL, 128, HC, S], bf16, kind="ExternalInput")
    uat_d = nc.dram_tensor("uat", [128, UT, HC, 128], bf16, kind="ExternalInput")
    wat_d = nc.dram_tensor("wat", [128, HAC, U], bf16, kind="ExternalInput")
    dect_d = nc.dram_tensor("dect", [128, HAC, BL], bf16, kind="ExternalInput")
    v_d = nc.dram_tensor("v", [128, UT], f32, kind="ExternalInput")
    out_d = nc.dram_tensor("out", [BL, S], f32, kind="ExternalOutput")

    with TileContext(nc) as tc:
        with (
            tc.tile_pool(name="const", bufs=1) as const,
            tc.tile_pool(name="encp", bufs=2) as encp,
            tc.tile_pool(name="energy", bufs=4) as energy,
            tc.tile_pool(name="accp", bufs=3) as accp,
            tc.tile_pool(name="smax", bufs=2) as smax,
            tc.tile_pool(name="psum_main", bufs=4, space="PSUM") as psum_main,
            tc.tile_pool(name="psum_v", bufs=2, space="PSUM") as psum_v,
            tc.tile_pool(name="psum_d", bufs=2, space="PSUM") as psum_d,
        ):
            # ---- constant loads; order = DMA issue order (startup path) ----
            dect_sb = const.tile([128, HAC, BL], bf16)
            nc.sync.dma_start(out=dect_sb, in_=dect_d[:, :, :])
            v_sb = const.tile([128, UT], f32)
            nc.sync.dma_start(out=v_sb, in_=v_d[:, :])
            uat_sb = const.tile([128, UT, HC, 128], bf16)
            nc.sync.dma_start(out=uat_sb[:, 0], in_=uat_d[:, 0])

            ones_sb = const.tile([128, 1], bf16)
            nc.vector.memset(ones_sb, 1.0)
            id8 = const.tile([8, 8], f32)
            make_identity(nc, id8)

            # PE warm-up: dummy matmuls on zeroed scratch keep the PE busy
            # (and the clock ramp / HAM warm) while input DMAs stream in
            scr_m = const.tile([128, S], bf16)
            nc.vector.memset(scr_m, 0.0)
            pwarm = psum_v.tile([1, S], f32, tag="pvs")
            for _ in range(12):
                nc.tensor.matmul(pwarm, lhsT=ones_sb, rhs=scr_m,
                                 start=True, stop=True)

            # ---- D = dec-projection (+folded biases), dec.T stationary ----
            # pdb[b, u-half] accumulated over 9 chunks; weight loads are the
            # tiny 8-column dec.T so PE setup cost is negligible. Emitted
            # after b0's first matmul block so the PE is already warm and the
            # wat DMAs have had time to land.
            wat_sb = const.tile([128, HAC, U], bf16)
            dbu_sb = const.tile([8, U], f32)
            d_sb = const.tile([128, UT * BL], f32)

            def emit_d_matmuls():
                for half, pdb in enumerate(
                    (psum_d.tile([8, 512], f32, tag="pd", name=f"pdb{h}")
                     for h in range(2))
                ):
                    u0 = half * 512
                    for hc in range(HAC):
                        nc.tensor.matmul(
                            pdb, lhsT=dect_sb[:, hc, :],
                            rhs=wat_sb[:, hc, u0:u0 + 512],
                            start=(hc == 0), stop=(hc == HAC - 1),
                        )
                    nc.vector.tensor_copy(
                        out=dbu_sb[:, u0:u0 + 512], in_=pdb)

            def emit_d_transposes():
                # D[b, u] -> D_T[u-in-tile, ut*8+b] via PE transpose
                pd = psum_d.tile([128, UT * BL], f32, tag="pd")
                for ut in range(UT):
                    nc.tensor.transpose(
                        pd[:, ut * BL:(ut + 1) * BL],
                        in_=dbu_sb[:, ut * 128:(ut + 1) * 128],
                        identity=id8,
                    )
                nc.vector.tensor_copy(out=d_sb, in_=pd)

            def emit_vdot_softmax(bl, acc):
                # att logits: partition-sum of acc via ones-matmul, then
                # per-row softmax entirely on partition 0
                pvs = psum_v.tile([1, S], f32)
                nc.tensor.matmul(pvs, lhsT=ones_sb, rhs=acc,
                                 start=True, stop=True)
                mneg = smax.tile([1, 1], f32)
                nc.vector.reduce_max(out=mneg, in_=pvs, axis=X, negate=True)
                ex = smax.tile([1, S], f32)
                ssum = smax.tile([1, 1], f32)
                nc.scalar.activation(out=ex, in_=pvs, func=Exp, bias=mneg,
                                     scale=1.0, accum_out=ssum)
                rsum = smax.tile([1, 1], f32)
                nc.vector.reciprocal(out=rsum, in_=ssum)
                res = smax.tile([1, S], f32)
                nc.vector.tensor_scalar_mul(res, ex, rsum)
                nc.sync.dma_start(out=out_d[bl:bl + 1, :], in_=res)

            # ---- main loop over local batches ----
            pending = None
            for bl in range(BL):
                enc_t = encp.tile([128, HC, S], bf16)
                for q in range(4):
                    h0, h1 = q * (HC // 4), (q + 1) * (HC // 4)
                    nc.sync.dma_start(
                        out=enc_t[:, h0:h1, :], in_=enc_d[bl, :, h0:h1, :]
                    )
                if bl == 0:
                    # remaining startup DMAs, ordered to match PE consumption
                    nc.sync.dma_start(out=wat_sb[:, :, 0:512],
                                      in_=wat_d[:, :, 0:512])
                    nc.sync.dma_start(out=wat_sb[:, :, 512:1024],
                                      in_=wat_d[:, :, 512:1024])
                    for ut in range(1, UT):
                        nc.sync.dma_start(out=uat_sb[:, ut], in_=uat_d[:, ut])
                acc_box = [None]

                def epilogue(ut, ps, bl=bl, acc_box=acc_box):
                    en = energy.tile([128, S], bf16, name="en")
                    col = ut * BL + bl
                    nc.scalar.activation(
                        out=en, in_=ps, func=Tanh,
                        bias=d_sb[:, col:col + 1], scale=1.0,
                    )
                    if ut == 0:
                        acc_box[0] = accp.tile([128, S], bf16, name="acc")
                        nc.vector.tensor_scalar_mul(acc_box[0], en, v_sb[:, 0:1])
                    else:
                        nc.vector.scalar_tensor_tensor(
                            out=acc_box[0], in0=en, scalar=v_sb[:, ut:ut + 1],
                            in1=acc_box[0], op0=MULT, op1=ADD,
                        )

                deferred_ps = None
                for ut in range(UT):
                    ps = psum_main.tile([128, S], f32)
                    for hc in range(HC):
                        nc.tensor.matmul(
                            ps,
                            lhsT=uat_sb[:, ut, hc, :],
                            rhs=enc_t[:, hc, :],
                            start=(hc == 0), stop=(hc == HC - 1),
                        )
                    if bl == 0 and ut == 0:
                        # d_sb isn't written yet: defer ut0's epilogue until
                        # the transposes (program order = dependency order)
                        emit_d_matmuls()
                        deferred_ps = ps
                        continue
                    if bl == 0 and ut == 1:
                        # by now the DVE psum->sbuf copies of D are done
                        emit_d_transposes()
                        epilogue(0, deferred_ps)
                    epilogue(ut, ps)
                acc = acc_box[0]
                if pending is not None:
                    emit_vdot_softmax(*pending)
                pending = (bl, acc)
            emit_vdot_softmax(*pending)

    nc.finalize()
    return nc


def _get_nc():
    global _BUILT
    if _BUILT is None:
        _BUILT = _build_bass()
    return _BUILT


def _prep_inputs(encoder_hiddens, last_dec_hidden, Ua_w, Ua_b, Wa_w, Wa_b, v_w):
    """Host-side sharding + layout prep (transpose contraction dims onto
    partitions, cast matmul operands to bf16, fold biases into an extra
    contraction row)."""
    enc = np.asarray(encoder_hiddens, dtype=np.float32)
    dec = np.asarray(last_dec_hidden, dtype=np.float32)
    Ua_w = np.asarray(Ua_w, dtype=np.float32)
    Wa_w = np.asarray(Wa_w, dtype=np.float32)
    bias_u = (np.asarray(Ua_b, np.float32) + np.asarray(Wa_b, np.float32))

    # p-major: [S,B,H] -> [B, 128p, HC, S] bf16 so each partition's span
    # is contiguous in DRAM (one big permute copy; per-core slices are
    # then zero-copy views along axis 0)
    enc_pm = np.ascontiguousarray(
        enc.astype(BF16).transpose(2, 1, 0).reshape(HC, 128, B, S)
        .transpose(2, 1, 0, 3)
    )                                                          # [B,128,HC,S]

    uat = np.ascontiguousarray(
        Ua_w.T.reshape(HC, 128, UT, 128).transpose(1, 2, 0, 3)
    ).astype(BF16)                                             # [128,UT,HC,128]

    wat_aug = np.zeros((HA, U), np.float32)
    wat_aug[:HD] = Wa_w.T
    wat_aug[HD] = bias_u
    wat_aug = np.ascontiguousarray(
        wat_aug.reshape(HAC, 128, U).transpose(1, 0, 2)
    ).astype(BF16)                                             # [128, HAC, U]

    v_prep = np.ascontiguousarray(
        np.asarray(v_w, np.float32).reshape(UT, 128).T
    )                                                         # [128, UT] f32

    in_maps = []
    for c in range(NCORES):
        b0 = c * BL
        enc_c = enc_pm[b0:b0 + BL]                            # [BL,128,HC,S]

        dect_aug = np.zeros((HA, BL), np.float32)
        dect_aug[:HD] = dec[b0:b0 + BL, :].T
        dect_aug[HD] = 1.0
        dect_aug = np.ascontiguousarray(
            dect_aug.reshape(HAC, 128, BL).transpose(1, 0, 2)
        ).astype(BF16)                                        # [128,HAC,BL]

        in_maps.append({
            "enc": enc_c,
            "uat": uat,
            "wat": wat_aug,
            "dect": dect_aug,
            "v": v_prep,
        })
    return in_maps


def kernel_with_results(**inputs):
    from concourse.bass_utils import run_bass_kernel_spmd

    nc = _get_nc()
    in_maps = _prep_inputs(**inputs)
    res = run_bass_kernel_spmd(nc, in_maps, core_ids=list(range(NCORES)))
    out = np.concatenate(
        [res.results[c]["out"] for c in range(NCORES)], axis=0
    ).astype(np.float32)
    return out, res


def kernel(**inputs):
    out, _ = kernel_with_results(**inputs)
    return out


# revision 18
# speedup vs baseline: 1.0139x; 1.0049x over previous
"""Trainium2 Bass kernel for Bahdanau-style attention.

reference:
    x[s,b,u]  = enc[s,b,:] @ Ua_w[u,:] + Ua_b[u] + dec[b,:] @ Wa_w[u,:] + Wa_b[u]
    att[b,s]  = softmax_s( sum_u v[u] * tanh(x[s,b,u]) )

Sharding: data-parallel over batch. 8 cores x 8 batches each; weights
replicated. All shapes hardcoded (S=512, B=64, H=2048, U=1024).

Per-core device program (bf16 matmul operands, fp32 accumulation):
  1. D[b,u] = dec-projection with Ua_b+Wa_b folded in via an extra
     contraction row (dec.T stationary so weight loads are 8 columns),
     then PE-transposed to D_T[u, ut*8+b] so it can feed the ScalarE
     activation bias port.
  2. for each local batch b (rows are b-major so the dec projection is
     constant along the moving free dim):
       for each u-tile (8 of 128):
         psum[128u, 512s] = sum_hc UaT[hc,utile].T @ enc[hc, b-rows]
         energy = tanh(psum + D_T[:, col])        (ScalarE, bias port)
         acc   += energy * v[utile]               (DVE fused mul-add)
       att[1, 512] = ones.T @ acc                 (PE partition-sum)
       softmax over s on [1, 512], DMA out row b.
"""

import numpy as np
import ml_dtypes

BF16 = ml_dtypes.bfloat16

S = 512          # src len
B = 64           # global batch
H = 2048         # encoder hidden (2*HIDDEN)
HD = 1024        # decoder hidden
U = 1024         # attention units
NCORES = 8
BL = B // NCORES  # local batch per core = 8

HC = H // 128     # 16 h-chunks for main contraction
UT = U // 128     # 8 u-tiles
HA = HD + 128     # augmented dec contraction (1024 + bias row + pad) = 1152
HAC = HA // 128   # 9 chunks

_BUILT = None     # cache so repeated kernel() calls reuse the program


def _build_bass():
    import concourse.mybir as mybir
    from concourse import bacc
    from concourse.tile import TileContext
    from concourse.masks import make_identity

    f32 = mybir.dt.float32
    bf16 = mybir.dt.bfloat16
    Tanh = mybir.ActivationFunctionType.Tanh
    Exp = mybir.ActivationFunctionType.Exp
    X = mybir.AxisListType.X
    MULT = mybir.AluOpType.mult
    ADD = mybir.AluOpType.add

    nc = bacc.Bacc("TRN2", num_devices=NCORES)

    # p-major layouts: partition p's data is contiguous in DRAM, so DMAs
    # are 128 descriptors of multi-KB runs instead of thousands of 1KB rows
    enc_d = nc.dram_tensor("enc", [BL, 128, HC, S], bf16, kind="ExternalInput")
    uat_d = nc.dram_tensor("uat", [128, UT, HC, 128], bf16, kind="ExternalInput")
    wat_d = nc.dram_tensor("wat", [128, HAC, U], bf16, kind="ExternalInput")
    dect_d = nc.dram_tensor("dect", [128, HAC, BL], bf16, kind="ExternalInput")
    v_d = nc.dram_tensor("v", [128, UT], f32, kind="ExternalInput")
    vb_d = nc.dram_tensor("vb", [128, UT], bf16, kind="ExternalInput")
    out_d = nc.dram_tensor("out", [BL, S], f32, kind="ExternalOutput")

    with TileContext(nc) as tc:
        with (
            tc.tile_pool(name="const", bufs=1) as const,
            tc.tile_pool(name="encp", bufs=2) as encp,
            tc.tile_pool(name="energy", bufs=5) as energy,
            tc.tile_pool(name="accp", bufs=3) as accp,
            tc.tile_pool(name="smax", bufs=2) as smax,
            tc.tile_pool(name="psum_main", bufs=6, space="PSUM") as psum_main,
            tc.tile_pool(name="psum_v", bufs=2, space="PSUM") as psum_v,
        ):
            # ---- constant loads; order = DMA issue order (startup path) ----
            dect_sb = const.tile([128, HAC, BL], bf16)
            v_sb = const.tile([128, UT], f32)
            vb_sb = const.tile([128, UT], bf16)
            uat_sb = const.tile([128, UT, HC, 128], bf16)
            nc.sync.dma_start(out=uat_sb[:, 0], in_=uat_d[:, 0])

            ones_sb = const.tile([128, 1], bf16)
            nc.vector.memset(ones_sb, 1.0)
            id8 = const.tile([8, 8], f32)
            make_identity(nc, id8)

            # PE warm-up: dummy matmuls on zeroed scratch keep the PE busy
            # (and the clock ramp / HAM warm) while input DMAs stream in
            scr_m = const.tile([128, S], bf16)
            nc.vector.memset(scr_m, 0.0)
            # fire one tiny activation early so the ~1.3us ACT table load
            # (tanh/exp set) happens during the DMA wait, not at first use
            scr_a = const.tile([1, 1], f32)
            nc.scalar.activation(out=scr_a, in_=scr_m[0:1, 0:1], func=Tanh)
            pwarm = psum_v.tile([1, S], f32, tag="pvs")
            for _ in range(14):
                nc.tensor.matmul(pwarm, lhsT=ones_sb, rhs=scr_m,
                                 start=True, stop=True)

            # ---- D = dec-projection (+folded biases), dec.T stationary ----
            # pdb[b, u-half] accumulated over 9 chunks; weight loads are the
            # tiny 8-column dec.T so PE setup cost is negligible. Emitted
            # after b0's first matmul block so the PE is already warm and the
            # wat DMAs have had time to land.
            wat_sb = const.tile([128, HAC, U], bf16)
            dbu_sb = const.tile([8, U], f32)
            d_sb = const.tile([128, UT * BL], f32)

            def emit_d_matmuls():
                for half, pdb in enumerate(
                    (psum_v.tile([8, 512], f32, tag="pvs", name=f"pdb{h}")
                     for h in range(2))
                ):
                    u0 = half * 512
                    for hc in range(HAC):
                        nc.tensor.matmul(
                            pdb, lhsT=dect_sb[:, hc, :],
                            rhs=wat_sb[:, hc, u0:u0 + 512],
                            start=(hc == 0), stop=(hc == HAC - 1),
                        )
                    nc.vector.tensor_copy(
                        out=dbu_sb[:, u0:u0 + 512], in_=pdb)

            def emit_d_transposes():
                # D[b, u] -> D_T[u-in-tile, ut*8+b] via PE transpose
                pd = psum_v.tile([128, UT * BL], f32, tag="pvs")
                for ut in range(UT):
                    nc.tensor.transpose(
                        pd[:, ut * BL:(ut + 1) * BL],
                        in_=dbu_sb[:, ut * 128:(ut + 1) * 128],
                        identity=id8,
                    )
                nc.vector.tensor_copy(out=d_sb, in_=pd)

            def emit_softmax(bl, pvs):
                # no max-subtraction: |logits| <= sum|v| ~= 16, exp is safe
                # in fp32 and the reference softmax is algebraically equal
                ex = smax.tile([1, S], f32)
                ssum = smax.tile([1, 1], f32)
                nc.scalar.activation(out=ex, in_=pvs, func=Exp,
                                     scale=1.0, accum_out=ssum)
                rsum = smax.tile([1, 1], f32)
                nc.vector.reciprocal(out=rsum, in_=ssum)
                res = smax.tile([1, S], f32)
                nc.vector.tensor_scalar_mul(res, ex, rsum)
                nc.sync.dma_start(out=out_d[bl:bl + 1, :], in_=res)

            def emit_vdot_softmax(bl, acc):
                # att logits: partition-sum of acc via ones-matmul, then
                # per-row softmax entirely on partition 0
                pvs = psum_v.tile([1, S], f32)
                nc.tensor.matmul(pvs, lhsT=ones_sb, rhs=acc,
                                 start=True, stop=True)
                emit_softmax(bl, pvs)

            # ---- main loop over local batches ----
            enc_ts = [None] * BL

            def dma_encq(bl, q):
                if enc_ts[bl] is None:
                    enc_ts[bl] = encp.tile([128, HC, S], bf16, name="enc_t")
                h0, h1 = q * (HC // 4), (q + 1) * (HC // 4)
                nc.sync.dma_start(
                    out=enc_ts[bl][:, h0:h1, :], in_=enc_d[bl, :, h0:h1, :]
                )

            # startup DMA order, interleaved to match PE consumption: uat
            # tiles arrive just before their u-tile block, wat before the D
            # matmuls, and batch 1's quarters stream during batch 0's tail
            dma_encq(0, 0)
            dma_encq(0, 1)
            nc.sync.dma_start(out=uat_sb[:, 1], in_=uat_d[:, 1])
            dma_encq(0, 2)
            dma_encq(0, 3)
            nc.sync.dma_start(out=uat_sb[:, 2], in_=uat_d[:, 2])
            nc.sync.dma_start(out=uat_sb[:, 3], in_=uat_d[:, 3])
            nc.sync.dma_start(out=wat_sb[:, :, 0:512], in_=wat_d[:, :, 0:512])
            nc.sync.dma_start(out=uat_sb[:, 4], in_=uat_d[:, 4])
            nc.sync.dma_start(out=wat_sb[:, :, 512:1024],
                              in_=wat_d[:, :, 512:1024])
            nc.sync.dma_start(out=uat_sb[:, 5], in_=uat_d[:, 5])
            nc.sync.dma_start(out=dect_sb, in_=dect_d[:, :, :])
            nc.sync.dma_start(out=v_sb, in_=v_d[:, :])
            nc.sync.dma_start(out=vb_sb, in_=vb_d[:, :])
            nc.sync.dma_start(out=uat_sb[:, 6], in_=uat_d[:, 6])
            dma_encq(1, 0)
            nc.sync.dma_start(out=uat_sb[:, 7], in_=uat_d[:, 7])
            dma_encq(1, 1)
            dma_encq(1, 2)
            dma_encq(1, 3)

            pending = None
            pv7 = None
            for bl in range(BL):
                enc_t = enc_ts[bl]
                acc_box = [None]

                def epilogue(ut, ps, bl=bl, acc_box=acc_box):
                    en = energy.tile([128, S], bf16, name="en")
                    col = ut * BL + bl
                    nc.scalar.activation(
                        out=en, in_=ps, func=Tanh,
                        bias=d_sb[:, col:col + 1], scale=1.0,
                    )
                    if bl == BL - 1:
                        return en  # last batch reduces on PE instead
                    if ut == 0:
                        acc_box[0] = accp.tile([128, S], bf16, name="acc")
                        nc.vector.tensor_scalar_mul(acc_box[0], en, v_sb[:, 0:1])
                    else:
                        nc.vector.scalar_tensor_tensor(
                            out=acc_box[0], in0=en, scalar=v_sb[:, ut:ut + 1],
                            in1=acc_box[0], op0=MULT, op1=ADD,
                        )
                    return en

                def emit_pe_vdot(pv7, en, ut):
                    if pv7 is None:
                        pv7 = psum_v.tile([1, S], f32, tag="pvs", name="pv7")
                    nc.tensor.matmul(pv7, lhsT=vb_sb[:, ut:ut + 1], rhs=en,
                                     start=(ut == 0), stop=(ut == UT - 1))
                    return pv7

                deferred = []
                ens = {}
                for ut in range(UT):
                    ps = psum_main.tile([128, S], f32)
                    for hc in range(HC):
                        nc.tensor.matmul(
                            ps,
                            lhsT=uat_sb[:, ut, hc, :],
                            rhs=enc_t[:, hc, :],
                            start=(hc == 0), stop=(hc == HC - 1),
                        )
                    if bl == 0 and ut <= 3:
                        # d_sb isn't written until the transposes: defer the
                        # epilogues (program order = dependency order). The
                        # wat DMAs land while ut0..3 compute.
                        deferred.append((ut, ps))
                        if ut == 3:
                            emit_d_matmuls()
                        continue
                    if bl == 0 and ut == 4:
                        emit_d_transposes()
                        for dut, dps in deferred:
                            ens[dut] = epilogue(dut, dps)
                    if ut == 2 and pending is not None:
                        # previous batch's logits: emitted two MM blocks into
                        # this batch so the PE never waits on the DVE chain
                        emit_vdot_softmax(*pending)
                        pending = None
                    if ut == 0 and 1 <= bl < BL - 1:
                        for q in range(4):
                            dma_encq(bl + 1, q)
                    ens[ut] = epilogue(ut, ps)
                    if bl == BL - 1 and ut >= 1:
                        # last batch: v-dot on PE, interleaved one block
                        # behind so the PE never waits on ACT
                        pv7 = emit_pe_vdot(pv7 if ut > 1 else None,
                                           ens.pop(ut - 1), ut - 1)
                acc = acc_box[0]
                if bl < BL - 1:
                    pending = (bl, acc)
            pv7 = emit_pe_vdot(pv7, ens.pop(UT - 1), UT - 1)
            emit_softmax(BL - 1, pv7)

    nc.finalize()
    return nc


def _get_nc():
    global _BUILT
    if _BUILT is None:
        _BUILT = _build_bass()
    return _BUILT


def _prep_inputs(encoder_hiddens, last_dec_hidden, Ua_w, Ua_b, Wa_w, Wa_b, v_w):
    """Host-side sharding + layout prep (transpose contraction dims onto
    partitions, cast matmul operands to bf16, fold biases into an extra
    contraction row)."""
    enc = np.asarray(encoder_hiddens, dtype=np.float32)
    dec = np.asarray(last_dec_hidden, dtype=np.float32)
    Ua_w = np.asarray(Ua_w, dtype=np.float32)
    Wa_w = np.asarray(Wa_w, dtype=np.float32)
    bias_u = (np.asarray(Ua_b, np.float32) + np.asarray(Wa_b, np.float32))

    # p-major: [S,B,H] -> [B, 128p, HC, S] bf16 so each partition's span
    # is contiguous in DRAM (one big permute copy; per-core slices are
    # then zero-copy views along axis 0)
    enc_pm = np.ascontiguousarray(
        enc.astype(BF16).transpose(2, 1, 0).reshape(HC, 128, B, S)
        .transpose(2, 1, 0, 3)
    )                                                          # [B,128,HC,S]

    uat = np.ascontiguousarray(
        Ua_w.T.reshape(HC, 128, UT, 128).transpose(1, 2, 0, 3)
    ).astype(BF16)                                             # [128,UT,HC,128]

    wat_aug = np.zeros((HA, U), np.float32)
    wat_aug[:HD] = Wa_w.T
    wat_aug[HD] = bias_u
    wat_aug = np.ascontiguousarray(
        wat_aug.reshape(HAC, 128, U).transpose(1, 0, 2)
    ).astype(BF16)                                             # [128, HAC, U]

    v_prep = np.ascontiguousarray(
        np.asarray(v_w, np.float32).reshape(UT, 128).T
    )                                                         # [128, UT] f32
    vb_prep = v_prep.astype(BF16)

    in_maps = []
    for c in range(NCORES):
        b0 = c * BL
        enc_c = enc_pm[b0:b0 + BL]                            # [BL,128,HC,S]

        dect_aug = np.zeros((HA, BL), np.float32)
        dect_aug[:HD] = dec[b0:b0 + BL, :].T
        dect_aug[HD] = 1.0
        dect_aug = np.ascontiguousarray(
            dect_aug.reshape(HAC, 128, BL).transpose(1, 0, 2)
        ).astype(BF16)                                        # [128,HAC,BL]

        in_maps.append({
            "enc": enc_c,
            "uat": uat,
            "wat": wat_aug,
            "dect": dect_aug,
            "v": v_prep,
            "vb": vb_prep,
        })
    return in_maps


def kernel_with_results(**inputs):
    from concourse.bass_utils import run_bass_kernel_spmd

    nc = _get_nc()
    in_maps = _prep_inputs(**inputs)
    res = run_bass_kernel_spmd(nc, in_maps, core_ids=list(range(NCORES)))
    out = np.concatenate(
        [res.results[c]["out"] for c in range(NCORES)], axis=0
    ).astype(np.float32)
    return out, res


def kernel(**inputs):
    out, _ = kernel_with_results(**inputs)
    return out


# revision 19
# speedup vs baseline: 1.0293x; 1.0152x over previous
"""Trainium2 Bass kernel for Bahdanau-style attention.

reference:
    x[s,b,u]  = enc[s,b,:] @ Ua_w[u,:] + Ua_b[u] + dec[b,:] @ Wa_w[u,:] + Wa_b[u]
    att[b,s]  = softmax_s( sum_u v[u] * tanh(x[s,b,u]) )

Sharding: data-parallel over batch. 8 cores x 8 batches each; weights
replicated. All shapes hardcoded (S=512, B=64, H=2048, U=1024).

Per-core device program (bf16 matmul operands, fp32 accumulation):
  1. D[b,u] = dec-projection with Ua_b+Wa_b folded in via an extra
     contraction row (dec.T stationary so weight loads are 8 columns),
     then PE-transposed to D_T[u, ut*8+b] so it can feed the ScalarE
     activation bias port.
  2. for each local batch b (rows are b-major so the dec projection is
     constant along the moving free dim):
       for each u-tile (8 of 128):
         psum[128u, 512s] = sum_hc UaT[hc,utile].T @ enc[hc, b-rows]
         energy = tanh(psum + D_T[:, col])        (ScalarE, bias port)
         acc   += energy * v[utile]               (DVE fused mul-add)
       att[1, 512] = ones.T @ acc                 (PE partition-sum)
       softmax over s on [1, 512], DMA out row b.
"""

import numpy as np
import ml_dtypes

BF16 = ml_dtypes.bfloat16

S = 512          # src len
B = 64           # global batch
H = 2048         # encoder hidden (2*HIDDEN)
HD = 1024        # decoder hidden
U = 1024         # attention units
NCORES = 8
BL = B // NCORES  # local batch per core = 8

HC = H // 128     # 16 h-chunks for main contraction
UT = U // 128     # 8 u-tiles
HA = HD + 128     # augmented dec contraction (1024 + bias row + pad) = 1152
HAC = HA // 128   # 9 chunks

_BUILT = None     # cache so repeated kernel() calls reuse the program


def _build_bass():
    import concourse.mybir as mybir
    from concourse import bacc
    from concourse.tile import TileContext
    from concourse.masks import make_identity

    f32 = mybir.dt.float32
    bf16 = mybir.dt.bfloat16
    Tanh = mybir.ActivationFunctionType.Tanh
    Exp = mybir.ActivationFunctionType.Exp
    MULT = mybir.AluOpType.mult
    ADD = mybir.AluOpType.add

    nc = bacc.Bacc("TRN2", num_devices=NCORES)

    # p-major layouts: partition p's data is contiguous in DRAM, so DMAs
    # are 128 descriptors of multi-KB runs instead of thousands of 1KB rows
    enc_d = nc.dram_tensor("enc", [BL, 128, HC, S], bf16, kind="ExternalInput")
    uat_d = nc.dram_tensor("uat", [128, UT, HC, 128], bf16, kind="ExternalInput")
    wat_d = nc.dram_tensor("wat", [128, HAC, U], bf16, kind="ExternalInput")
    dect_d = nc.dram_tensor("dect", [128, HAC, BL], bf16, kind="ExternalInput")
    v_d = nc.dram_tensor("v", [128, UT], f32, kind="ExternalInput")
    vb_d = nc.dram_tensor("vb", [128, UT], bf16, kind="ExternalInput")
    out_d = nc.dram_tensor("out", [BL, S], f32, kind="ExternalOutput")

    with TileContext(nc) as tc:
        with (
            tc.tile_pool(name="const", bufs=1) as const,
            tc.tile_pool(name="encp", bufs=2) as encp,
            tc.tile_pool(name="energy", bufs=5) as energy,
            tc.tile_pool(name="accp", bufs=3) as accp,
            tc.tile_pool(name="smax", bufs=2) as smax,
            tc.tile_pool(name="psum_main", bufs=6, space="PSUM") as psum_main,
            tc.tile_pool(name="psum_v", bufs=2, space="PSUM") as psum_v,
        ):
            # ---- constant loads; order = DMA issue order (startup path) ----
            dect_sb = const.tile([128, HAC, BL], bf16)
            v_sb = const.tile([128, UT], f32)
            vb_sb = const.tile([128, UT], bf16)
            uat_sb = const.tile([128, UT, HC, 128], bf16)
            nc.sync.dma_start(out=uat_sb[:, 0], in_=uat_d[:, 0])

            ones_sb = const.tile([128, 1], bf16)
            nc.vector.memset(ones_sb, 1.0)
            id8 = const.tile([8, 8], f32)
            make_identity(nc, id8)

            # PE warm-up: dummy matmuls on zeroed scratch keep the PE busy
            # (and the clock ramp / HAM warm) while input DMAs stream in
            scr_m = const.tile([128, S], bf16)
            nc.vector.memset(scr_m, 0.0)
            # fire one tiny activation early so the ~1.3us ACT table load
            # (tanh/exp set) happens during the DMA wait, not at first use
            scr_a = const.tile([1, 1], f32)
            nc.scalar.activation(out=scr_a, in_=scr_m[0:1, 0:1], func=Tanh)
            pwarm = psum_v.tile([1, S], f32, tag="pvs")
            for _ in range(14):
                nc.tensor.matmul(pwarm, lhsT=ones_sb, rhs=scr_m,
                                 start=True, stop=True)

            # ---- D = dec-projection (+folded biases), dec.T stationary ----
            # pdb[b, u-half] accumulated over 9 chunks; weight loads are the
            # tiny 8-column dec.T so PE setup cost is negligible. Emitted
            # after b0's first matmul block so the PE is already warm and the
            # wat DMAs have had time to land.
            wat_sb = const.tile([128, HAC, U], bf16)
            dbu_sb = const.tile([8, U], f32)
            d_sb = const.tile([128, UT * BL], f32)

            def emit_d_matmuls():
                for half, pdb in enumerate(
                    (psum_v.tile([8, 512], f32, tag="pvs", name=f"pdb{h}")
                     for h in range(2))
                ):
                    u0 = half * 512
                    for hc in range(HAC):
                        nc.tensor.matmul(
                            pdb, lhsT=dect_sb[:, hc, :],
                            rhs=wat_sb[:, hc, u0:u0 + 512],
                            start=(hc == 0), stop=(hc == HAC - 1),
                        )
                    nc.vector.tensor_copy(
                        out=dbu_sb[:, u0:u0 + 512], in_=pdb)

            def emit_d_transposes():
                # D[b, u] -> D_T[u-in-tile, ut*8+b] via PE transpose
                pd = psum_v.tile([128, UT * BL], f32, tag="pvs")
                for ut in range(UT):
                    nc.tensor.transpose(
                        pd[:, ut * BL:(ut + 1) * BL],
                        in_=dbu_sb[:, ut * 128:(ut + 1) * 128],
                        identity=id8,
                    )
                nc.vector.tensor_copy(out=d_sb, in_=pd)

            def emit_softmax(bl, pvs):
                # no max-subtraction: |logits| <= sum|v| ~= 16, exp is safe
                # in fp32 and the reference softmax is algebraically equal
                ex = smax.tile([1, S], f32)
                ssum = smax.tile([1, 1], f32)
                nc.scalar.activation(out=ex, in_=pvs, func=Exp,
                                     scale=1.0, accum_out=ssum)
                rsum = smax.tile([1, 1], f32)
                nc.vector.reciprocal(out=rsum, in_=ssum)
                res = smax.tile([1, S], f32)
                nc.vector.tensor_scalar_mul(res, ex, rsum)
                nc.sync.dma_start(out=out_d[bl:bl + 1, :], in_=res)

            def emit_vdot_softmax(bl, acc):
                # att logits: partition-sum of acc via ones-matmul, then
                # per-row softmax entirely on partition 0
                pvs = psum_v.tile([1, S], f32)
                nc.tensor.matmul(pvs, lhsT=ones_sb, rhs=acc,
                                 start=True, stop=True)
                emit_softmax(bl, pvs)

            # ---- main loop over local batches ----
            enc_ts = [None] * BL

            def dma_encq(bl, q):
                if enc_ts[bl] is None:
                    enc_ts[bl] = encp.tile([128, HC, S], bf16, name="enc_t")
                h0, h1 = q * (HC // 4), (q + 1) * (HC // 4)
                nc.sync.dma_start(
                    out=enc_ts[bl][:, h0:h1, :], in_=enc_d[bl, :, h0:h1, :]
                )

            # startup DMA order, interleaved to match PE consumption: uat
            # tiles arrive just before their u-tile block, wat before the D
            # matmuls, and batch 1's quarters stream during batch 0's tail
            dma_encq(0, 0)
            dma_encq(0, 1)
            nc.sync.dma_start(out=uat_sb[:, 1], in_=uat_d[:, 1])
            dma_encq(0, 2)
            dma_encq(0, 3)
            nc.sync.dma_start(out=uat_sb[:, 2], in_=uat_d[:, 2])
            nc.sync.dma_start(out=uat_sb[:, 3], in_=uat_d[:, 3])
            nc.sync.dma_start(out=wat_sb[:, :, 0:512], in_=wat_d[:, :, 0:512])
            nc.sync.dma_start(out=uat_sb[:, 4], in_=uat_d[:, 4])
            nc.sync.dma_start(out=wat_sb[:, :, 512:1024],
                              in_=wat_d[:, :, 512:1024])
            nc.sync.dma_start(out=uat_sb[:, 5], in_=uat_d[:, 5])
            nc.sync.dma_start(out=dect_sb, in_=dect_d[:, :, :])
            nc.sync.dma_start(out=v_sb, in_=v_d[:, :])
            nc.sync.dma_start(out=vb_sb, in_=vb_d[:, :])
            nc.sync.dma_start(out=uat_sb[:, 6], in_=uat_d[:, 6])
            dma_encq(1, 0)
            nc.sync.dma_start(out=uat_sb[:, 7], in_=uat_d[:, 7])
            dma_encq(1, 1)
            dma_encq(1, 2)
            dma_encq(1, 3)

            pending = None
            pv7 = None
            for bl in range(BL):
                enc_t = enc_ts[bl]
                acc_box = [None]

                def epilogue(ut, ps, bl=bl, acc_box=acc_box):
                    en = energy.tile([128, S], bf16, name="en")
                    col = ut * BL + bl
                    nc.scalar.activation(
                        out=en, in_=ps, func=Tanh,
                        bias=d_sb[:, col:col + 1], scale=1.0,
                    )
                    if bl == BL - 1:
                        return en  # last batch reduces on PE instead
                    if ut == 0:
                        acc_box[0] = accp.tile([128, S], bf16, name="acc")
                        nc.vector.tensor_scalar_mul(acc_box[0], en, v_sb[:, 0:1])
                    else:
                        nc.vector.scalar_tensor_tensor(
                            out=acc_box[0], in0=en, scalar=v_sb[:, ut:ut + 1],
                            in1=acc_box[0], op0=MULT, op1=ADD,
                        )
                    return en

                def emit_pe_vdot(pv7, en, ut):
                    if pv7 is None:
                        pv7 = psum_v.tile([1, S], f32, tag="pvs", name="pv7")
                    nc.tensor.matmul(pv7, lhsT=vb_sb[:, ut:ut + 1], rhs=en,
                                     start=(ut == 0), stop=(ut == UT - 1))
                    return pv7

                deferred = []
                ens = {}
                for ut in range(UT):
                    ps = psum_main.tile([128, S], f32)
                    for hc in range(HC):
                        nc.tensor.matmul(
                            ps,
                            lhsT=uat_sb[:, ut, hc, :],
                            rhs=enc_t[:, hc, :],
                            start=(hc == 0), stop=(hc == HC - 1),
                        )
                    if bl == 0 and ut <= 3:
                        # d_sb isn't written until the transposes: defer the
                        # epilogues (program order = dependency order). The
                        # wat DMAs land while ut0..3 compute.
                        deferred.append((ut, ps))
                        if ut == 3:
                            emit_d_matmuls()
                        continue
                    if bl == 0 and ut == 4:
                        emit_d_transposes()
                        for dut, dps in deferred:
                            ens[dut] = epilogue(dut, dps)
                    if ut == 2 and pending is not None:
                        # previous batch's logits: emitted two MM blocks into
                        # this batch so the PE never waits on the DVE chain
                        emit_vdot_softmax(*pending)
                        pending = None
                    if ut == 0 and 1 <= bl < BL - 1:
                        for q in range(4):
                            dma_encq(bl + 1, q)
                    ens[ut] = epilogue(ut, ps)
                    if bl == BL - 1 and ut >= 1:
                        # last batch: v-dot on PE, interleaved one block
                        # behind so the PE never waits on ACT
                        pv7 = emit_pe_vdot(pv7 if ut > 1 else None,
                                           ens.pop(ut - 1), ut - 1)
                acc = acc_box[0]
                if bl < BL - 1:
                    pending = (bl, acc)
            pv7 = emit_pe_vdot(pv7, ens.pop(UT - 1), UT - 1)
            emit_softmax(BL - 1, pv7)

    nc.finalize()
    return nc


def _get_nc():
    global _BUILT
    if _BUILT is None:
        _BUILT = _build_bass()
    return _BUILT


def _prep_inputs(encoder_hiddens, last_dec_hidden, Ua_w, Ua_b, Wa_w, Wa_b, v_w):
    """Host-side sharding + layout prep (transpose contraction dims onto
    partitions, cast matmul operands to bf16, fold biases into an extra
    contraction row)."""
    enc = np.asarray(encoder_hiddens, dtype=np.float32)
    dec = np.asarray(last_dec_hidden, dtype=np.float32)
    Ua_w = np.asarray(Ua_w, dtype=np.float32)
    Wa_w = np.asarray(Wa_w, dtype=np.float32)
    bias_u = (np.asarray(Ua_b, np.float32) + np.asarray(Wa_b, np.float32))

    # p-major: [S,B,H] -> [B, 128p, HC, S] bf16 so each partition's span
    # is contiguous in DRAM (one big permute copy; per-core slices are
    # then zero-copy views along axis 0)
    enc_pm = np.ascontiguousarray(
        enc.astype(BF16).transpose(2, 1, 0).reshape(HC, 128, B, S)
        .transpose(2, 1, 0, 3)
    )                                                          # [B,128,HC,S]

    uat = np.ascontiguousarray(
        Ua_w.T.reshape(HC, 128, UT, 128).transpose(1, 2, 0, 3)
    ).astype(BF16)                                             # [128,UT,HC,128]

    wat_aug = np.zeros((HA, U), np.float32)
    wat_aug[:HD] = Wa_w.T
    wat_aug[HD] = bias_u
    wat_aug = np.ascontiguousarray(
        wat_aug.reshape(HAC, 128, U).transpose(1, 0, 2)
    ).astype(BF16)                                             # [128, HAC, U]

    v_prep = np.ascontiguousarray(
        np.asarray(v_w, np.float32).reshape(UT, 128).T
    )                                                         # [128, UT] f32
    vb_prep = v_prep.astype(BF16)

    in_maps = []
    for c in range(NCORES):
        b0 = c * BL
        enc_c = enc_pm[b0:b0 + BL]                            # [BL,128,HC,S]

        dect_aug = np.zeros((HA, BL), np.float32)
        dect_aug[:HD] = dec[b0:b0 + BL, :].T
        dect_aug[HD] = 1.0
        dect_aug = np.ascontiguousarray(
            dect_aug.reshape(HAC, 128, BL).transpose(1, 0, 2)
        ).astype(BF16)                                        # [128,HAC,BL]

        in_maps.append({
            "enc": enc_c,
            "uat": uat,
            "wat": wat_aug,
            "dect": dect_aug,
            "v": v_prep,
            "vb": vb_prep,
        })
    return in_maps


def kernel_with_results(**inputs):
    from concourse.bass_utils import run_bass_kernel_spmd

    nc = _get_nc()
    in_maps = _prep_inputs(**inputs)
    res = run_bass_kernel_spmd(nc, in_maps, core_ids=list(range(NCORES)))
    out = np.concatenate(
        [res.results[c]["out"] for c in range(NCORES)], axis=0
    ).astype(np.float32)
    return out, res


def kernel(**inputs):
    out, _ = kernel_with_results(**inputs)
    return out
